# revision 13
# baseline (speedup 1.0000x reference)
"""Causal linear attention (Katharopoulos et al.) Trainium2 Bass kernel.

Problem: B=4, L=2048, H=8, D=64, f32.
  Q' = elu(Q)+1, K' = elu(K)+1
  out[b,l,h,:] = (sum_{i<=l} (Q'[l].K'[i]) V[i]) / (Q'[l].cumsum(K')[l] + eps)

Sharding: 8 cores, core c <- batch b=c//2, head-quad hq=c%2 (4 heads).
Per-core problem: q,k,v [2048, 4*64] -> o [2048, 4*64].

Algorithm (chunked linear attention, chunk C=128, per head p):
  AT[i,l]  = sum_d K'[i,d] Q'[l,d]                             (PE, contraction d=64)
  ATm      = AT * uppertri(incl)                               (DVE)
  Y1[l,:]  = ATm.T @ V1_t + Q'_t @ S_{t-1}   (V1 = [V | 1])    (PE, accum in PSUM)
  S_t      = S_{t-1} + K'_t^T @ V1_t                           (PE, accum in PSUM)
  out      = Y1[:, :64] * (1 / Y1[:, 64])                      (DVE)
"""

import os
import numpy as np

import concourse.bass as bass
import concourse.bacc as bacc
import concourse.tile as tile
from concourse import mybir
from concourse.bass_utils import run_bass_kernel_spmd
from concourse.masks import make_identity, make_upper_triangular

F32 = mybir.dt.float32
B, L, H, D = 4, 2048, 8, 64
N_CORES = 8
HPC = 4          # heads per core
W = HPC * D      # 256 free-dim width per core
C = 128          # sequence chunk
T = L // C       # 16 chunks
AluOp = mybir.AluOpType
Act = mybir.ActivationFunctionType


def _ap(t, offset_elems, dims):
    """Build an AP on tile t with explicit [step, count] dims (incl partition)."""
    base = t[:] if not isinstance(t, bass.AP) else t
    return bass.AP(tensor=base.tensor, offset=base.offset + offset_elems, ap=dims)


def build_bass(stage: int = 0) -> bass.Bass:
    """stage: 0=full, 1=dma roundtrip, 2=feature map, 3=transposes, 4=AT+mask,
    5=no inter/S (chunk-local attention only)."""
    nc = bacc.Bacc(None, target_bir_lowering=False, debug=False)
    q_d = nc.dram_tensor("q", [L, W], F32, kind="ExternalInput")
    k_d = nc.dram_tensor("k", [L, W], F32, kind="ExternalInput")
    v_d = nc.dram_tensor("v", [L, W], F32, kind="ExternalInput")
    o_d = nc.dram_tensor("o", [L, W], F32, kind="ExternalOutput")

    with tile.TileContext(nc) as tc:
        with (
            tc.tile_pool(name="consts", bufs=1) as consts,
            tc.tile_pool(name="state", bufs=1) as state,
            tc.tile_pool(name="inp", bufs=3) as inp,
            tc.tile_pool(name="work", bufs=2) as work,
            tc.tile_pool(name="outp", bufs=2) as outp,
            tc.tile_pool(name="qt_ps", bufs=1, space="PSUM") as qt_pool,
            tc.tile_pool(name="kt_ps", bufs=1, space="PSUM") as kt_pool,
            tc.tile_pool(name="at_ps", bufs=2, space="PSUM") as at_pool,
            tc.tile_pool(name="y1_ps", bufs=2, space="PSUM") as y1_pool,
            tc.tile_pool(name="s_ps", bufs=1, space="PSUM") as s_pool,
        ):
            ident = consts.tile([128, 128], F32)
            make_identity(nc, ident)
            mask4 = consts.tile([128, 4, 128], F32)
            make_upper_triangular(nc, mask4[:, 0, :], val=1.0, diag=True)
            for g in range(1, 4):
                nc.gpsimd.tensor_copy(out=mask4[:, g, :], in_=mask4[:, 0, :])

            # persistent state: S in PSUM (accumulated), S_sb snapshot in SBUF
            s_ps = s_pool.tile([64, 4, 65], F32)
            nc.vector.memset(s_ps, 0.0)
            s_sb = state.tile([64, 4, 65], F32)

            for t in range(T):
                r0 = t * C
                # ---- loads ----
                qn = inp.tile([128, W], F32)
                nc.sync.dma_start(out=qn, in_=q_d[r0 : r0 + C, :])
                kn = inp.tile([128, W], F32)
                nc.sync.dma_start(out=kn, in_=k_d[r0 : r0 + C, :])
                v1 = inp.tile([128, 4, 65], F32)
                nc.sync.dma_start(
                    out=v1[:, :, 0:64],
                    in_=v_d[r0 : r0 + C, :].rearrange("p (h d) -> p h d", d=64),
                )
                nc.gpsimd.memset(v1[:, :, 64], 1.0)

                if stage == 1:
                    nc.sync.dma_start(
                        out=o_d[r0 : r0 + C, :].rearrange("p (h d) -> p h d", d=64),
                        in_=v1[:, :, 0:64],
                    )
                    continue

                # ---- feature map: fm(x) = max(x,0) + exp(min(x,0)) ----
                qm = work.tile([128, W], F32)
                nc.gpsimd.tensor_scalar_min(out=qm, in0=qn, scalar1=0.0)
                qe = work.tile([128, W], F32)
                nc.scalar.activation(out=qe, in_=qm, func=Act.Exp)
                qp = work.tile([128, W], F32)
                nc.vector.scalar_tensor_tensor(
                    out=qp, in0=qn, scalar=0.0, in1=qe,
                    op0=AluOp.max, op1=AluOp.add,
                )
                km = work.tile([128, W], F32)
                nc.gpsimd.tensor_scalar_min(out=km, in0=kn, scalar1=0.0)
                ke = work.tile([128, W], F32)
                nc.scalar.activation(out=ke, in_=km, func=Act.Exp)
                kp = work.tile([128, W], F32)
                nc.vector.scalar_tensor_tensor(
                    out=kp, in0=kn, scalar=0.0, in1=ke,
                    op0=AluOp.max, op1=AluOp.add,
                )

                if stage == 2:
                    nc.sync.dma_start(out=o_d[r0 : r0 + C, :], in_=qp)
                    continue

                # ---- per-head transposes (PE, via identity): [128,64] -> [64,128]
                qt_ps = qt_pool.tile([64, 4, 128], F32)
                kt_ps = kt_pool.tile([64, 4, 128], F32)
                for p in range(4):
                    nc.tensor.matmul(
                        out=qt_ps[:, p, :],
                        lhsT=qp[:, p * 64 : p * 64 + 64], rhs=ident,
                    )
                for p in range(4):
                    nc.tensor.matmul(
                        out=kt_ps[:, p, :],
                        lhsT=kp[:, p * 64 : p * 64 + 64], rhs=ident,
                    )
                qt = work.tile([64, 4, 128], F32)
                nc.scalar.copy(out=qt, in_=qt_ps)
                kt = work.tile([64, 4, 128], F32)
                nc.vector.tensor_copy(out=kt, in_=kt_ps)

                if stage == 3:
                    nc.sync.dma_start(
                        out=o_d[r0 : r0 + C, :],
                        in_=qt[:].rearrange("p h d -> p (h d)")[:, 0:256],
                    )
                    continue

                # ---- AT = K' Q'^T  (per head; contraction d=64) ----
                at_ps = at_pool.tile([128, 512], F32)
                for p in range(4):
                    nc.tensor.matmul(
                        out=at_ps[:, p * 128 : p * 128 + 128],
                        lhsT=kt[:, p, :],
                        rhs=qt[:, p, :],
                    )
                atm = work.tile([128, 512], F32)
                nc.vector.tensor_mul(
                    out=atm, in0=at_ps,
                    in1=mask4[:].rearrange("p h d -> p (h d)"),
                )

                if stage == 4:
                    nc.sync.dma_start(out=o_d[r0 : r0 + C, :], in_=atm[:, 0:256])
                    continue

                # ---- Y1 = intra + inter  (accumulate per-head groups) ----
                y1_ps = y1_pool.tile([128, 4, 65], F32)
                for p in range(4):
                    nc.tensor.matmul(
                        out=y1_ps[:, p, :],
                        lhsT=atm[:, p * 128 : p * 128 + 128],
                        rhs=v1[:, p, :],
                        start=True, stop=(t == 0 or stage == 5),
                    )
                    if t > 0 and stage != 5:
                        nc.tensor.matmul(
                            out=y1_ps[:, p, :],
                            lhsT=qt[:, p, :],
                            rhs=s_sb[:, p, :],
                            start=False, stop=True,
                        )

                # ---- S += K'^T V1 ----
                if stage != 5:
                    for p in range(4):
                        nc.tensor.matmul(
                            out=s_ps[:, p, :],
                            lhsT=kp[:, p * 64 : p * 64 + 64],
                            rhs=v1[:, p, :],
                            start=False, stop=(t == T - 1),
                            skip_group_check=True,
                        )
                    if t < T - 1:
                        nc.scalar.copy(out=s_sb, in_=s_ps)

                # ---- normalize + store ----
                zr = outp.tile([128, 4], F32)
                nc.vector.reciprocal(out=zr, in_=y1_ps[:, :, 64])
                ot = outp.tile([128, 4, 64], F32)
                zr_b = _ap(zr, 0, [zr[:].ap[0], [1, 4], [0, 64]])
                nc.vector.tensor_mul(out=ot, in0=y1_ps[:, :, 0:64], in1=zr_b)
                nc.sync.dma_start(
                    out=o_d[r0 : r0 + C, :].rearrange("p (h d) -> p h d", d=64),
                    in_=ot,
                )
    nc.compile()
    return nc


_nc_cache = None


def _get_nc():
    global _nc_cache
    if _nc_cache is None:
        _nc_cache = build_bass()
    return _nc_cache


def kernel(queries: np.ndarray, keys: np.ndarray, values: np.ndarray) -> np.ndarray:
    nc = _get_nc()
    in_maps = []
    for c in range(N_CORES):
        b, hq = c // 2, c % 2
        hs = slice(hq * HPC, (hq + 1) * HPC)
        in_maps.append({
            "q": np.ascontiguousarray(np.asarray(queries)[b, :, hs, :]).reshape(L, W),
            "k": np.ascontiguousarray(np.asarray(keys)[b, :, hs, :]).reshape(L, W),
            "v": np.ascontiguousarray(np.asarray(values)[b, :, hs, :]).reshape(L, W),
        })
    res = run_bass_kernel_spmd(nc, in_maps, core_ids=list(range(N_CORES))).results
    out = np.empty((B, L, H, D), dtype=np.float32)
    for c in range(N_CORES):
        b, hq = c // 2, c % 2
        out[b, :, hq * HPC : (hq + 1) * HPC, :] = res[c]["o"].reshape(L, HPC, D)
    return out


# revision 19
# speedup vs baseline: 2.1876x; 2.1876x over previous
"""Causal linear attention (Katharopoulos et al.) Trainium2 Bass kernel.

Problem: B=4, L=2048, H=8, D=64, f32.
  Q' = elu(Q)+1, K' = elu(K)+1
  out[b,l,h,:] = (sum_{i<=l} (Q'[l].K'[i]) V[i]) / (Q'[l].cumsum(K')[l] + eps)

Sharding: 8 cores, core c <- batch b=c//2, head-quad hq=c%2 (4 heads).
Per-core problem: q,k,v [2048, 4*64] -> o [2048, 4*64].

fp16 PE datapath (single-pass matmuls), f32 PSUM accumulation, f32
normalization. Software-pipelined with a 2-chunk skew so the PE never
waits on same-chunk elementwise work:
  iteration t emits: transposes(t) | AT+mask(t-1) | Y1+S+normalize(t-2)
Loads + feature map are batched over groups of 4 chunks.
"""

import numpy as np

import concourse.bass as bass
import concourse.bacc as bacc
import concourse.tile as tile
from concourse import mybir
from concourse.bass_utils import run_bass_kernel_spmd
from concourse.masks import make_identity, make_upper_triangular

F32 = mybir.dt.float32
F16 = mybir.dt.float16
B, L, H, D = 4, 2048, 8, 64
N_CORES = 8
HPC = 4          # heads per core
W = HPC * D      # 256
C = 128          # sequence chunk
T = L // C       # 16 chunks
G = 4            # chunks per load/fm group
NG = T // G
AluOp = mybir.AluOpType
Act = mybir.ActivationFunctionType


def _ap(t, offset_elems, dims):
    base = t[:] if not isinstance(t, bass.AP) else t
    return bass.AP(tensor=base.tensor, offset=base.offset + offset_elems, ap=dims)


def build_bass() -> bass.Bass:
    nc = bacc.Bacc(None, target_bir_lowering=False, debug=False)
    q_d = nc.dram_tensor("q", [L, W], F32, kind="ExternalInput")
    k_d = nc.dram_tensor("k", [L, W], F32, kind="ExternalInput")
    v_d = nc.dram_tensor("v", [L, W], F32, kind="ExternalInput")
    o_d = nc.dram_tensor("o", [L, W], F32, kind="ExternalOutput")

    with tile.TileContext(nc) as tc:
        with (
            tc.tile_pool(name="consts", bufs=1) as consts,
            tc.tile_pool(name="state", bufs=1) as state,
            tc.tile_pool(name="ldq", bufs=2) as ldq,
            tc.tile_pool(name="ldv", bufs=3) as ldv,
            tc.tile_pool(name="fmp", bufs=2) as fmp,
            tc.tile_pool(name="tws", bufs=3) as tws,
            tc.tile_pool(name="work", bufs=2) as work,
            tc.tile_pool(name="outp", bufs=2) as outp,
            tc.tile_pool(name="qt_ps", bufs=2, space="PSUM") as qt_pool,
            tc.tile_pool(name="kt_ps", bufs=1, space="PSUM") as kt_pool,
            tc.tile_pool(name="at_ps", bufs=2, space="PSUM") as at_pool,
            tc.tile_pool(name="y1_ps", bufs=2, space="PSUM") as y1_pool,
            tc.tile_pool(name="s_ps", bufs=1, space="PSUM") as s_pool,
        ):
            ident = consts.tile([128, 128], F16)
            make_identity(nc, ident)
            mask4 = consts.tile([128, 4, 128], F16)
            make_upper_triangular(nc, mask4[:, 0, :], val=1.0, diag=True)
            for g in range(1, 4):
                nc.gpsimd.tensor_copy(out=mask4[:, g, :], in_=mask4[:, 0, :])

            s_ps = s_pool.tile([64, 4, 65], F32)
            nc.vector.memset(s_ps, 0.0)
            s_sb = state.tile([64, 4, 65], F16)

            qk4 = {}   # group -> [128, 2, G, W] f16   (0=q, 1=k)
            v14 = {}   # group -> [128, G, 4, 65] f16  (ones col at 64)
            fp4 = {}   # group -> [128, 2, G, W] f16   (Q', K')
            qts = {}   # chunk -> [64, 4, 128] f16
            kts = {}
            atms = {}  # chunk -> [128, 512] f16
            v1s = {}   # chunk -> AP view [128, 4, 65]
            kps = {}   # chunk -> AP view of K' natural [128, W]

            def emit_load(g):
                r0 = g * G * C
                qk = ldq.tile([128, 2, G, W], F16, name=f"qk4_{g}", tag="qk4")
                src = lambda dram: dram[r0 : r0 + G * C, :].rearrange(
                    "(t p) w -> p t w", p=C)
                nc.gpsimd.dma_start(out=qk[:, 0, :, :], in_=src(q_d))
                nc.gpsimd.dma_start(out=qk[:, 1, :, :], in_=src(k_d))
                v1 = ldv.tile([128, G, 4, 65], F16, name=f"v14_{g}", tag="v14")
                for tt_ in range(G):
                    nc.gpsimd.dma_start(
                        out=v1[:, tt_, :, 0:64],
                        in_=v_d[r0 + tt_ * C : r0 + (tt_ + 1) * C, :].rearrange(
                            "p (h d) -> p h d", d=64),
                    )
                nc.gpsimd.memset(v1[:, :, :, 64], 1.0)
                qk4[g] = qk
                v14[g] = v1

            def emit_fm(g):
                qk = qk4[g]
                mn = work.tile([128, 2, G, W], F16, name=f"mn_{g}", tag="mn")
                nc.vector.tensor_scalar_min(out=mn, in0=qk, scalar1=0.0)
                ex = work.tile([128, 2, G, W], F16, name=f"ex_{g}", tag="ex")
                nc.scalar.activation(out=ex, in_=mn, func=Act.Exp)
                rl = work.tile([128, 2, G, W], F16, name=f"rl_{g}", tag="rl")
                nc.vector.tensor_scalar_max(out=rl, in0=qk, scalar1=0.0)
                fp = fmp.tile([128, 2, G, W], F16, name=f"fp4_{g}", tag="fp4")
                nc.vector.tensor_add(out=fp, in0=ex, in1=rl)
                fp4[g] = fp
                for tt_ in range(G):
                    t_ = g * G + tt_
                    v1s[t_] = v14[g][:, tt_, :, :]
                    kps[t_] = fp[:, 1, tt_, :]

            def emit_transpose(t):
                g, tt_ = t // G, t % G
                fp = fp4[g]
                qt_ps = qt_pool.tile([64, 4, 128], F32, name=f"qtps_{t}", tag="qtps")
                kt_ps = kt_pool.tile([64, 4, 128], F32, name=f"ktps_{t}", tag="ktps")
                for p in range(4):
                    nc.tensor.matmul(
                        out=qt_ps[:, p, :],
                        lhsT=fp[:, 0, tt_, p * 64 : p * 64 + 64], rhs=ident)
                for p in range(4):
                    nc.tensor.matmul(
                        out=kt_ps[:, p, :],
                        lhsT=fp[:, 1, tt_, p * 64 : p * 64 + 64], rhs=ident)
                qt = tws.tile([64, 4, 128], F16, name=f"qt_{t}", tag="qt")
                nc.scalar.copy(out=qt, in_=qt_ps)
                kt = tws.tile([64, 4, 128], F16, name=f"kt_{t}", tag="kt")
                nc.scalar.copy(out=kt[:, 0:2, :], in_=kt_ps[:, 0:2, :])
                nc.vector.tensor_copy(out=kt[:, 2:4, :], in_=kt_ps[:, 2:4, :])
                qts[t] = qt
                kts[t] = kt

            def emit_at(t):
                at_ps = at_pool.tile([128, 512], F32, name=f"atps_{t}", tag="atps")
                for p in range(4):
                    nc.tensor.matmul(
                        out=at_ps[:, p * 128 : p * 128 + 128],
                        lhsT=kts[t][:, p, :],
                        rhs=qts[t][:, p, :])
                atm = work.tile([128, 512], F16, name=f"atm_{t}", tag="atm")
                nc.vector.tensor_mul(
                    out=atm, in0=at_ps,
                    in1=mask4[:].rearrange("p h d -> p (h d)"))
                atms[t] = atm
                del kts[t]

            def emit_tail(t):
                y1_ps = y1_pool.tile([128, 4, 65], F32, name=f"y1_{t}", tag="y1")
                for p in range(4):
                    nc.tensor.matmul(
                        out=y1_ps[:, p, :],
                        lhsT=atms[t][:, p * 128 : p * 128 + 128],
                        rhs=v1s[t][:, p, :],
                        start=True, stop=(t == 0))
                    if t > 0:
                        nc.tensor.matmul(
                            out=y1_ps[:, p, :],
                            lhsT=qts[t][:, p, :],
                            rhs=s_sb[:, p, :],
                            start=False, stop=True)
                for p in range(4):
                    nc.tensor.matmul(
                        out=s_ps[:, p, :],
                        lhsT=kps[t][:, p * 64 : p * 64 + 64],
                        rhs=v1s[t][:, p, :],
                        start=False, stop=(t == T - 1),
                        skip_group_check=True)
                if t < T - 1:
                    nc.scalar.copy(out=s_sb[:, 0:2, :], in_=s_ps[:, 0:2, :])
                    nc.vector.tensor_copy(out=s_sb[:, 2:4, :], in_=s_ps[:, 2:4, :])

                zr = outp.tile([128, 4], F32, name=f"zr_{t}", tag="zr")
                nc.vector.reciprocal(out=zr, in_=y1_ps[:, :, 64])
                ot = outp.tile([128, 4, 64], F32, name=f"ot_{t}", tag="ot")
                zr_b = _ap(zr, 0, [zr[:].ap[0], [1, 4], [0, 64]])
                nc.vector.tensor_mul(out=ot, in0=y1_ps[:, :, 0:64], in1=zr_b)
                nc.sync.dma_start(
                    out=o_d[t * C : t * C + C, :].rearrange(
                        "p (h d) -> p h d", d=64),
                    in_=ot)
                del atms[t], qts[t]

            # prologue: prefetch two groups
            emit_load(0)
            emit_load(1)
            for t in range(T + 2):
                if t < T:
                    g, tt_ = t // G, t % G
                    if tt_ == 0:
                        if g >= 1 and g + 1 < NG:
                            emit_load(g + 1)
                        emit_fm(g)
                    emit_transpose(t)
                if 1 <= t <= T:
                    emit_at(t - 1)
                if 2 <= t:
                    emit_tail(t - 2)
    nc.compile()
    return nc


_nc_cache = None


def _get_nc():
    global _nc_cache
    if _nc_cache is None:
        _nc_cache = build_bass()
    return _nc_cache


def kernel(queries: np.ndarray, keys: np.ndarray, values: np.ndarray) -> np.ndarray:
    nc = _get_nc()
    in_maps = []
    for c in range(N_CORES):
        b, hq = c // 2, c % 2
        hs = slice(hq * HPC, (hq + 1) * HPC)
        in_maps.append({
            "q": np.ascontiguousarray(np.asarray(queries)[b, :, hs, :]).reshape(L, W),
            "k": np.ascontiguousarray(np.asarray(keys)[b, :, hs, :]).reshape(L, W),
            "v": np.ascontiguousarray(np.asarray(values)[b, :, hs, :]).reshape(L, W),
        })
    res = run_bass_kernel_spmd(nc, in_maps, core_ids=list(range(N_CORES))).results
    out = np.empty((B, L, H, D), dtype=np.float32)
    for c in range(N_CORES):
        b, hq = c // 2, c % 2
        out[b, :, hq * HPC : (hq + 1) * HPC, :] = res[c]["o"].reshape(L, HPC, D)
    return out
